# revision 1
# baseline (speedup 1.0000x reference)
"""LoRA layer kernel for Trainium2 (8 NeuronCores, data-parallel).

Computes out = SCALING * (x @ A^T) @ B^T for x [4, 8192, 1024],
lora_A [4, 1024], lora_B [1024, 4], SCALING = 0.25.

Strategy (per core, shard = 4096 rows x 1024 features):
  - x rows are sharded across the 8 cores; A/B replicated (host pre-arranged).
  - Per 512-row group: DMA x in natural layout, transpose 128x128 chunks on
    the PE (fp32r transpose mode) into PSUM, evacuate to SBUF with the DVE,
    rank-4 matmuls (fp32r, N=512) for h^T = A x^T, then out = h @ (0.25 B^T)
    with rows back on partitions so the store is contiguous; ScalarE
    evacuates the output PSUM banks; one 2 MiB DMA store per group.
"""

import sys

for _p in (
    "/root/.axon_site",
    "/root/.axon_site/_ro/trn_rl_repo",
    "/root/.axon_site/_ro/pypackages",
):
    if _p not in sys.path:
        sys.path.insert(0, _p)

from contextlib import ExitStack

import numpy as np

N_CORES = 8
D_IN = 1024
D_OUT = 1024
RANK = 4
ROWS_TOTAL = 4 * 8192
ROWS_PER_CORE = ROWS_TOTAL // N_CORES  # 4096
SCALING = 1.0 / RANK

P = 128          # partitions
GROUP_ROWS = 512  # rows processed per pipeline iteration (4 subtiles of 128)
N_CHUNKS = D_IN // P  # 8 feature chunks


def emit_lora(tc, x_ap, at_ap, bt_ap, id_ap, out_ap, rows):
    """Emit the LoRA kernel IR for one core's shard of `rows` rows.

    x_ap  : DRAM [rows, D_IN]  f32 (declared f32r; raw f32 bits)
    at_ap : DRAM [P, N_CHUNKS, RANK] f32r, at[p, c, r] = A[r, c*P + p]
    bt_ap : DRAM [RANK, D_OUT] f32r, bt[r, o] = SCALING * B[o, r]
    id_ap : DRAM [P, P] f32r identity (for PE transpose)
    out_ap: DRAM [rows, D_OUT] f32
    """
    import concourse.mybir as mybir

    nc = tc.nc
    f32 = mybir.dt.float32
    f32r = mybir.dt.float32r
    ctx = tc._ctx  # ExitStack owned by caller

    n_groups = rows // GROUP_ROWS
    J = GROUP_ROWS // P  # 4 row subtiles per group
    n_ochunks = D_OUT // 512  # 2 output column chunks of 512

    consts = ctx.enter_context(tc.tile_pool(name="consts", bufs=1))
    xpool = ctx.enter_context(tc.tile_pool(name="xin", bufs=4))
    xtpool = ctx.enter_context(tc.tile_pool(name="xt", bufs=8))
    htpool = ctx.enter_context(tc.tile_pool(name="ht", bufs=2))
    opool = ctx.enter_context(tc.tile_pool(name="osb", bufs=3))
    ps_xt = ctx.enter_context(tc.tile_pool(name="ps_xt", bufs=3, space="PSUM"))
    ps_ht = ctx.enter_context(tc.tile_pool(name="ps_ht", bufs=2, space="PSUM"))
    ps_o = ctx.enter_context(tc.tile_pool(name="ps_o", bufs=3, space="PSUM"))

    # rows -> partitions: row = n*P + p
    x_pnm = x_ap.rearrange("(n p) m -> p n m", p=P)
    o_pnm = out_ap.rearrange("(n p) m -> p n m", p=P)

    # First group's x loads lead the HWDGE ring; the tiny constants ride the
    # SWDGE ring in parallel so nothing delays the first transposes.
    x_sb0 = xpool.tile([P, J, D_IN], f32r)
    for j in range(J):
        nc.sync.dma_start(x_sb0[:, j, :], x_pnm[:, j, :])

    ident = consts.tile([P, P], f32r)
    nc.gpsimd.dma_start(ident[:], id_ap[:])
    at_sb = consts.tile([P, N_CHUNKS, RANK], f32r)
    nc.gpsimd.dma_start(at_sb[:], at_ap[:])
    bt_sb = consts.tile([RANK, D_OUT], f32r)
    nc.gpsimd.dma_start(bt_sb[:], bt_ap[:])

    for g in range(n_groups):
        if g == 0:
            x_sb = x_sb0
        else:
            x_sb = xpool.tile([P, J, D_IN], f32r)
            for j in range(J):
                nc.sync.dma_start(x_sb[:, j, :], x_pnm[:, g * J + j, :])

        ht_ps = ps_ht.tile([RANK, GROUP_ROWS], f32)
        for c in range(N_CHUNKS):
            # Transpose the 4 row-subtiles of feature chunk c into one PSUM
            # bank: xt_ps[p=feat, j, m=row] = x[row, feat]. One accumulation
            # group per bank (start on first write, stop on last).
            xt_ps = ps_xt.tile([P, J, P], f32r)
            for j in range(J):
                nc.tensor.matmul(
                    xt_ps[:, j, :],
                    lhsT=x_sb[:, j, c * P : (c + 1) * P],
                    rhs=ident[:],
                    is_transpose=True,
                    start=(j == 0),
                    stop=(j == J - 1),
                )
            xt_sb = xtpool.tile([P, J, P], f32r)
            nc.vector.tensor_copy(xt_sb[:], xt_ps[:])
            # h^T[r, m] += sum_f A^T[cP+f, r] * xT[f, m]
            nc.tensor.matmul(
                ht_ps[:],
                lhsT=at_sb[:, c, :],
                rhs=xt_sb[:],
                start=(c == 0),
                stop=(c == N_CHUNKS - 1),
            )

        ht_sb = htpool.tile([RANK, GROUP_ROWS], f32r)
        nc.vector.tensor_copy(ht_sb[:], ht_ps[:])

        o_sb = opool.tile([P, J, D_OUT], f32)
        for j in range(J):
            for o2 in range(n_ochunks):
                o_ps = ps_o.tile([P, 512], f32)
                # out[m, o] = sum_r h^T[r, m] * bt[r, o]
                nc.tensor.matmul(
                    o_ps[:],
                    lhsT=ht_sb[:, j * P : (j + 1) * P],
                    rhs=bt_sb[:, o2 * 512 : (o2 + 1) * 512],
                    start=True,
                    stop=True,
                )
                nc.scalar.copy(o_sb[:, j, o2 * 512 : (o2 + 1) * 512], o_ps[:])

            # Stores ride the SWDGE (gpsimd) ring so a store waiting on its
            # copy never head-of-line-blocks the HWDGE load ring.
            nc.gpsimd.dma_start(o_pnm[:, g * J + j, :], o_sb[:, j, :])


def build_nc(rows=ROWS_PER_CORE):
    import concourse.mybir as mybir
    import concourse.tile as tile
    from concourse import bacc

    f32 = mybir.dt.float32
    f32r = mybir.dt.float32r
    nc = bacc.Bacc("TRN2", target_bir_lowering=False, debug=False)
    x_d = nc.dram_tensor("x", [rows, D_IN], f32r, kind="ExternalInput").ap()
    at_d = nc.dram_tensor("at", [P, N_CHUNKS, RANK], f32r, kind="ExternalInput").ap()
    bt_d = nc.dram_tensor("bt", [RANK, D_OUT], f32r, kind="ExternalInput").ap()
    id_d = nc.dram_tensor("ident", [P, P], f32r, kind="ExternalInput").ap()
    out_d = nc.dram_tensor("out", [rows, D_OUT], f32, kind="ExternalOutput").ap()

    with tile.TileContext(nc) as tc:
        with ExitStack() as ctx:
            tc._ctx = ctx
            emit_lora(tc, x_d, at_d, bt_d, id_d, out_d, rows)
    nc.compile()
    return nc


def round_tf32(a):
    """Round f32 to tfloat32 (10-bit mantissa), round-to-nearest-even."""
    u = np.ascontiguousarray(a, dtype=np.float32).view(np.uint32)
    r = (u + 0x0FFF + ((u >> 13) & 1)) & np.uint32(0xFFFFE000)
    return r.view(np.float32)


def host_prep(lora_A, lora_B):
    # at[p, c, r] = A[r, c*P + p]
    at = np.ascontiguousarray(
        lora_A.T.reshape(N_CHUNKS, P, RANK).transpose(1, 0, 2), dtype=np.float32
    )
    bt = np.ascontiguousarray(lora_B.T * SCALING, dtype=np.float32)
    return round_tf32(at), round_tf32(bt)


_NC_CACHE = {}


def kernel(x, lora_A, lora_B):
    from concourse.bass_utils import run_bass_kernel_spmd

    if "nc" not in _NC_CACHE:
        _NC_CACHE["nc"] = build_nc(ROWS_PER_CORE)
    nc = _NC_CACHE["nc"]

    x2 = np.ascontiguousarray(x, dtype=np.float32).reshape(ROWS_TOTAL, D_IN)
    at, bt = host_prep(np.asarray(lora_A), np.asarray(lora_B))
    ident = np.eye(P, dtype=np.float32)
    shards = x2.reshape(N_CORES, ROWS_PER_CORE, D_IN)
    in_maps = [
        {"x": np.ascontiguousarray(shards[i]), "at": at, "bt": bt, "ident": ident}
        for i in range(N_CORES)
    ]
    res = run_bass_kernel_spmd(nc, in_maps, core_ids=list(range(N_CORES)))
    out = np.concatenate([res.results[i]["out"] for i in range(N_CORES)], axis=0)
    return out.reshape(4, 8192, D_OUT)



# revision 2
# speedup vs baseline: 1.5013x; 1.5013x over previous
"""LoRA layer kernel for Trainium2 (8 NeuronCores, data-parallel).

Computes out = SCALING * (x @ A^T) @ B^T for x [4, 8192, 1024],
lora_A [4, 1024], lora_B [1024, 4], SCALING = 0.25.

Strategy (per core, shard = 4096 rows x 1024 features), fp16 wire format:
  - x rows sharded across 8 cores; A/B replicated.
  - The host pre-casts x to fp16 and pre-transposes each shard to a
    feature-major grouped layout [P=128, G=8, C=8, M=512] so every DMA
    load lands 8 KiB contiguous per partition and the kernel needs no
    on-chip transposes at all (the f32 baseline spent ~60% of PE time
    on 128x128 PE transposes and was DMA-bound moving f32).
  - Per 512-row group: one 1 MiB load (SP HWDGE ring), 8 rank-4
    accumulating matmuls h^T = A x^T (PSUM f32), evac to fp16, then
    8 output matmuls out = h @ (0.25 B^T) with rows on partitions;
    PSUM evacuation alternates DVE/ScalarE; one 1 MiB fp16 store per
    group on the ACT HWDGE ring so loads and stores drain from
    independent queues.
  - Host converts the fp16 result back to f32 and un-permutes rows.
"""

import sys

for _p in (
    "/root/.axon_site",
    "/root/.axon_site/_ro/trn_rl_repo",
    "/root/.axon_site/_ro/pypackages",
):
    if _p not in sys.path:
        sys.path.insert(0, _p)

from contextlib import ExitStack

import numpy as np

N_CORES = 8
D_IN = 1024
D_OUT = 1024
RANK = 4
ROWS_TOTAL = 4 * 8192
ROWS_PER_CORE = ROWS_TOTAL // N_CORES  # 4096
SCALING = 1.0 / RANK

P = 128            # partitions
C = D_IN // P      # 8 feature chunks
GROUP_ROWS = 512   # rows per pipeline iteration
N_GROUPS = ROWS_PER_CORE // GROUP_ROWS  # 8
J = GROUP_ROWS // P  # 4 row subtiles per group
OCH = 512          # output columns per PSUM bank


def emit_lora(tc, x_ap, at_ap, bt_ap, out_ap):
    """Emit the LoRA kernel IR for one core's shard.

    x_ap  : DRAM [P, N_GROUPS, C, GROUP_ROWS] fp16,
            x_ap[p, g, c, m] = x[g*512 + m, c*128 + p]
    at_ap : DRAM [P, C, RANK] fp16, at[p, c, r] = A[r, c*128 + p]
    bt_ap : DRAM [RANK, D_OUT] fp16, bt[r, o] = SCALING * B[o, r]
    out_ap: DRAM [P, N_GROUPS, J, D_OUT] fp16, row = g*512 + j*128 + p
    """
    import concourse.mybir as mybir

    nc = tc.nc
    f32 = mybir.dt.float32
    f16 = mybir.dt.float16
    ctx = tc._ctx  # ExitStack owned by caller

    consts = ctx.enter_context(tc.tile_pool(name="consts", bufs=1))
    xtpool = ctx.enter_context(tc.tile_pool(name="xt", bufs=4))
    htpool = ctx.enter_context(tc.tile_pool(name="ht", bufs=2))
    opool = ctx.enter_context(tc.tile_pool(name="osb", bufs=3))
    ps_ht = ctx.enter_context(tc.tile_pool(name="ps_ht", bufs=2, space="PSUM"))
    ps_o = ctx.enter_context(tc.tile_pool(name="ps_o", bufs=4, space="PSUM"))

    # First group's x load leads the SP HWDGE ring; the tiny constants ride
    # the SWDGE ring in parallel so nothing delays the first matmuls.
    xt0 = xtpool.tile([P, C, GROUP_ROWS], f16)
    nc.sync.dma_start(xt0[:], x_ap[:, 0])

    at_sb = consts.tile([P, C, RANK], f16)
    nc.gpsimd.dma_start(at_sb[:], at_ap[:])
    bt_sb = consts.tile([RANK, D_OUT], f16)
    nc.gpsimd.dma_start(bt_sb[:], bt_ap[:])

    for g in range(N_GROUPS):
        if g == 0:
            xt = xt0
        else:
            xt = xtpool.tile([P, C, GROUP_ROWS], f16)
            nc.sync.dma_start(xt[:], x_ap[:, g])

        # h^T[r, m] += sum_f A^T[c*128+f, r] * x^T[c*128+f, m]
        ht_ps = ps_ht.tile([RANK, GROUP_ROWS], f32)
        for c in range(C):
            nc.tensor.matmul(
                ht_ps[:],
                lhsT=at_sb[:, c, :],
                rhs=xt[:, c, :],
                start=(c == 0),
                stop=(c == C - 1),
            )
        ht_sb = htpool.tile([RANK, GROUP_ROWS], f16)
        nc.vector.tensor_copy(ht_sb[:], ht_ps[:])

        o_sb = opool.tile([P, J, D_OUT], f16)
        for j in range(J):
            for o2 in range(D_OUT // OCH):
                o_ps = ps_o.tile([P, OCH], f32)
                # out[m, o] = sum_r h^T[r, m] * bt[r, o]
                nc.tensor.matmul(
                    o_ps[:],
                    lhsT=ht_sb[:, j * P : (j + 1) * P],
                    rhs=bt_sb[:, o2 * OCH : (o2 + 1) * OCH],
                    start=True,
                    stop=True,
                )
                dst = o_sb[:, j, o2 * OCH : (o2 + 1) * OCH]
                if (j * 2 + o2) % 2 == 0:
                    nc.vector.tensor_copy(dst, o_ps[:])
                else:
                    nc.scalar.copy(dst, o_ps[:])

        # Stores ride the ACT HWDGE ring; loads own the SP ring.
        nc.scalar.dma_start(out_ap[:, g], o_sb[:])


def build_nc():
    import concourse.mybir as mybir
    import concourse.tile as tile
    from concourse import bacc

    f16 = mybir.dt.float16
    nc = bacc.Bacc("TRN2", target_bir_lowering=False, debug=False)
    x_d = nc.dram_tensor(
        "x", [P, N_GROUPS, C, GROUP_ROWS], f16, kind="ExternalInput"
    ).ap()
    at_d = nc.dram_tensor("at", [P, C, RANK], f16, kind="ExternalInput").ap()
    bt_d = nc.dram_tensor("bt", [RANK, D_OUT], f16, kind="ExternalInput").ap()
    out_d = nc.dram_tensor(
        "out", [P, N_GROUPS, J, D_OUT], f16, kind="ExternalOutput"
    ).ap()

    with tile.TileContext(nc) as tc:
        with ExitStack() as ctx:
            tc._ctx = ctx
            emit_lora(tc, x_d, at_d, bt_d, out_d)
    nc.compile()
    return nc


def host_prep(lora_A, lora_B):
    # at[p, c, r] = A[r, c*P + p]
    at = np.ascontiguousarray(
        np.asarray(lora_A, dtype=np.float32).T.reshape(C, P, RANK).transpose(1, 0, 2)
    ).astype(np.float16)
    bt = (np.asarray(lora_B, dtype=np.float32).T * SCALING).astype(np.float16)
    return at, np.ascontiguousarray(bt)


def shard_x(x):
    """x [4, 8192, 1024] f32 -> per-core [P, G, C, M] fp16 feature-major."""
    x2 = np.asarray(x).astype(np.float16).reshape(N_CORES, ROWS_PER_CORE, D_IN)
    shards = []
    for i in range(N_CORES):
        xt = x2[i].T  # [D_IN, rows] ; xt[c*128+p, g*512+m]
        xdev = xt.reshape(C, P, N_GROUPS, GROUP_ROWS).transpose(1, 2, 0, 3)
        shards.append(np.ascontiguousarray(xdev))
    return shards


def unshard_out(results):
    """Per-core out [P, G, J, D_OUT] fp16 -> full [4, 8192, 1024] f32."""
    outs = []
    for r in results:
        o = r["out"]  # [P, N_GROUPS, J, D_OUT] fp16 ; row = g*512 + j*128 + p
        outs.append(o.transpose(1, 2, 0, 3).reshape(ROWS_PER_CORE, D_OUT))
    return np.concatenate(outs, axis=0).astype(np.float32).reshape(4, 8192, D_OUT)


_NC_CACHE = {}


def kernel(x, lora_A, lora_B):
    from concourse.bass_utils import run_bass_kernel_spmd

    if "nc" not in _NC_CACHE:
        _NC_CACHE["nc"] = build_nc()
    nc = _NC_CACHE["nc"]

    shards = shard_x(x)
    at, bt = host_prep(lora_A, lora_B)
    in_maps = [{"x": shards[i], "at": at, "bt": bt} for i in range(N_CORES)]
    res = run_bass_kernel_spmd(nc, in_maps, core_ids=list(range(N_CORES)))
    return unshard_out([res.results[i] for i in range(N_CORES)])


# revision 3
# speedup vs baseline: 1.5735x; 1.0481x over previous
"""LoRA layer kernel for Trainium2 (8 NeuronCores, data-parallel).

Computes out = SCALING * (x @ A^T) @ B^T for x [4, 8192, 1024],
lora_A [4, 1024], lora_B [1024, 4], SCALING = 0.25.

Strategy (per core, shard = 4096 rows x 1024 features), fp16 wire format:
  - x rows sharded across 8 cores; A/B replicated on every core.
  - The host pre-casts x to fp16 and pre-transposes each shard to a
    feature-major grouped layout [P=128, G=8, C=8, M=512] so every DMA
    load lands 8 KiB contiguous per partition and the kernel needs no
    on-chip transposes at all.
  - The rank-4 matrices are replicated 32x on the host so both matmul
    stages run [K=128, M=128, N=512] on a fully lit 128x128 PE array:
    a rank-4 lhsT (M=4) looks idle to the PE activity monitor, which
    re-throttles the PE clock to 1.2 GHz; padded to 128 it stays at
    2.4 GHz and fast-weight-load engages.  at_rep carries a 1/32 scale
    so the 32 redundant replicas sum back to the true product.
  - Per 512-row group: one 1 MiB load (SP HWDGE ring), 8 accumulating
    matmuls ht_rep = at_rep^T x^T, DVE evac to fp16, 8 output matmuls
    out = ht_rep^T bt_rep, PSUM evacuation alternating DVE/ScalarE,
    one 1 MiB fp16 store per group on the ACT HWDGE ring.  Constants
    also ride the ACT ring so nothing blocks the first group.
  - Host converts the fp16 result back to f32 and un-permutes rows.
"""

import sys

for _p in (
    "/root/.axon_site",
    "/root/.axon_site/_ro/trn_rl_repo",
    "/root/.axon_site/_ro/pypackages",
):
    if _p not in sys.path:
        sys.path.insert(0, _p)

from contextlib import ExitStack

import numpy as np

N_CORES = 8
D_IN = 1024
D_OUT = 1024
RANK = 4
REP = 32           # replicas of the rank-4 factors to fill 128 partitions
ROWS_TOTAL = 4 * 8192
ROWS_PER_CORE = ROWS_TOTAL // N_CORES  # 4096
SCALING = 1.0 / RANK

P = 128            # partitions
C = D_IN // P      # 8 feature chunks
GROUP_ROWS = 512   # rows per pipeline iteration
N_GROUPS = ROWS_PER_CORE // GROUP_ROWS  # 8
J = GROUP_ROWS // P  # 4 row subtiles per group
OCH = 512          # output columns per PSUM bank


def emit_lora(tc, x_ap, at_ap, bt_ap, out_ap):
    """Emit the LoRA kernel IR for one core's shard.

    x_ap  : DRAM [P, N_GROUPS, C, GROUP_ROWS] fp16,
            x_ap[p, g, c, m] = x[g*512 + m, c*128 + p]
    at_ap : DRAM [P, C, P] fp16, at[p, c, 32k+r] = A[r, c*128 + p] / 32
    bt_ap : DRAM [P, D_OUT] fp16, bt[32k+r, o] = SCALING * B[o, r]
    out_ap: DRAM [P, N_GROUPS, J, D_OUT] fp16, row = g*512 + j*128 + p
    """
    import concourse.mybir as mybir

    nc = tc.nc
    f32 = mybir.dt.float32
    f16 = mybir.dt.float16
    ctx = tc._ctx  # ExitStack owned by caller

    consts = ctx.enter_context(tc.tile_pool(name="consts", bufs=1))
    xtpool = ctx.enter_context(tc.tile_pool(name="xt", bufs=4))
    htpool = ctx.enter_context(tc.tile_pool(name="ht", bufs=2))
    opool = ctx.enter_context(tc.tile_pool(name="osb", bufs=3))
    ps_ht = ctx.enter_context(tc.tile_pool(name="ps_ht", bufs=2, space="PSUM"))
    ps_o = ctx.enter_context(tc.tile_pool(name="ps_o", bufs=4, space="PSUM"))

    # First group's x load leads the SP HWDGE ring; constants ride the ACT
    # HWDGE ring in parallel so nothing delays the first matmuls.
    xt0 = xtpool.tile([P, C, GROUP_ROWS], f16)
    nc.sync.dma_start(xt0[:], x_ap[:, 0])

    at_sb = consts.tile([P, C, P], f16)
    nc.scalar.dma_start(at_sb[:], at_ap[:])
    bt_sb = consts.tile([P, D_OUT], f16)
    nc.scalar.dma_start(bt_sb[:], bt_ap[:])

    for g in range(N_GROUPS):
        if g == 0:
            xt = xt0
        else:
            xt = xtpool.tile([P, C, GROUP_ROWS], f16)
            nc.sync.dma_start(xt[:], x_ap[:, g])

        # ht_rep[32k+r, m] += sum_f at_rep[c*128+f, 32k+r] * x^T[c*128+f, m]
        ht_ps = ps_ht.tile([P, GROUP_ROWS], f32)
        for c in range(C):
            nc.tensor.matmul(
                ht_ps[:],
                lhsT=at_sb[:, c, :],
                rhs=xt[:, c, :],
                start=(c == 0),
                stop=(c == C - 1),
            )
        ht_sb = htpool.tile([P, GROUP_ROWS], f16)
        nc.vector.tensor_copy(ht_sb[:], ht_ps[:])

        o_sb = opool.tile([P, J, D_OUT], f16)
        for j in range(J):
            for o2 in range(D_OUT // OCH):
                o_ps = ps_o.tile([P, OCH], f32)
                # out[m, o] = sum_{32k+r} ht_rep[32k+r, m] * bt_rep[32k+r, o]
                nc.tensor.matmul(
                    o_ps[:],
                    lhsT=ht_sb[:, j * P : (j + 1) * P],
                    rhs=bt_sb[:, o2 * OCH : (o2 + 1) * OCH],
                    start=True,
                    stop=True,
                )
                dst = o_sb[:, j, o2 * OCH : (o2 + 1) * OCH]
                if (j * 2 + o2) % 2 == 0:
                    nc.scalar.copy(dst, o_ps[:])
                else:
                    nc.vector.tensor_copy(dst, o_ps[:])

        # Stores ride the ACT HWDGE ring; loads own the SP ring.
        nc.scalar.dma_start(out_ap[:, g], o_sb[:])


def build_nc():
    import concourse.mybir as mybir
    import concourse.tile as tile
    from concourse import bacc

    f16 = mybir.dt.float16
    nc = bacc.Bacc("TRN2", target_bir_lowering=False, debug=False)
    x_d = nc.dram_tensor(
        "x", [P, N_GROUPS, C, GROUP_ROWS], f16, kind="ExternalInput"
    ).ap()
    at_d = nc.dram_tensor("at", [P, C, P], f16, kind="ExternalInput").ap()
    bt_d = nc.dram_tensor("bt", [P, D_OUT], f16, kind="ExternalInput").ap()
    out_d = nc.dram_tensor(
        "out", [P, N_GROUPS, J, D_OUT], f16, kind="ExternalOutput"
    ).ap()

    with tile.TileContext(nc) as tc:
        with ExitStack() as ctx:
            tc._ctx = ctx
            emit_lora(tc, x_d, at_d, bt_d, out_d)
    nc.compile()
    return nc


def host_prep(lora_A, lora_B):
    # at[p, c, 32k+r] = A[r, c*P + p] / REP  (REP identical replicas, scaled
    # so the redundant 32-fold sum in the output matmul is exact)
    a = np.asarray(lora_A, dtype=np.float32) / REP  # [RANK, D_IN]
    atc = a.T.reshape(C, P, RANK).transpose(1, 0, 2)  # [P, C, RANK]
    at = np.tile(atc, (1, 1, REP)).astype(np.float16)  # [P, C, RANK*REP]
    # bt[32k+r, o] = SCALING * B[o, r]
    b = (np.asarray(lora_B, dtype=np.float32).T * SCALING).astype(np.float16)
    bt = np.tile(b, (REP, 1))  # [P, D_OUT]
    return np.ascontiguousarray(at), np.ascontiguousarray(bt)


def shard_x(x):
    """x [4, 8192, 1024] f32 -> per-core [P, G, C, M] fp16 feature-major."""
    x2 = np.asarray(x).astype(np.float16).reshape(N_CORES, ROWS_PER_CORE, D_IN)
    shards = []
    for i in range(N_CORES):
        xt = x2[i].T  # [D_IN, rows] ; xt[c*128+p, g*512+m]
        xdev = xt.reshape(C, P, N_GROUPS, GROUP_ROWS).transpose(1, 2, 0, 3)
        shards.append(np.ascontiguousarray(xdev))
    return shards


def unshard_out(results):
    """Per-core out [P, G, J, D_OUT] fp16 -> full [4, 8192, 1024] f32."""
    outs = []
    for r in results:
        o = r["out"]  # [P, N_GROUPS, J, D_OUT] fp16 ; row = g*512 + j*128 + p
        outs.append(o.transpose(1, 2, 0, 3).reshape(ROWS_PER_CORE, D_OUT))
    return np.concatenate(outs, axis=0).astype(np.float32).reshape(4, 8192, D_OUT)


_NC_CACHE = {}


def kernel(x, lora_A, lora_B):
    from concourse.bass_utils import run_bass_kernel_spmd

    if "nc" not in _NC_CACHE:
        _NC_CACHE["nc"] = build_nc()
    nc = _NC_CACHE["nc"]

    shards = shard_x(x)
    at, bt = host_prep(lora_A, lora_B)
    in_maps = [{"x": shards[i], "at": at, "bt": bt} for i in range(N_CORES)]
    res = run_bass_kernel_spmd(nc, in_maps, core_ids=list(range(N_CORES)))
    return unshard_out([res.results[i] for i in range(N_CORES)])
